# revision 22
# baseline (speedup 1.0000x reference)
"""Trainium2 Bass kernel for nn_LocalLoadBalancingLoss (v3).

loss = mean_b var_l(u) + 0.5 * mean_b max_l(u),
u[b,l] = (sum_{t: link(t)=l} pred[b,t] * dem[b, t//8]) / (cap[l] + 1e-8)

Strategy (pure data parallel over batch, 8 cores x 8192 rows):
  Row-paired tiles: one tile = 256 rows laid out [128p, 2rr, ...] with
  partition p holding DRAM rows {base+2p, base+2p+1}; group DMAs move
  ~3.2 MB of contiguous >=6 KB descriptors per dma_start.

  Per 256-row tile (free width TW = 2*792 = 1584, padded to 1664):
    - DVE: tt(bf16) = pred * broadcast(dem)   (two ops, one per rr)
    - PE : 13x transpose of 128-wide tt chunks -> PSUM (bf16)
    - ACT: evacuate ttT PSUM -> SBUF (2 copies: 8-chunk + 5-chunk banks)
    - PE : 13x scatter matmul with the one-hot link mask as the
           STATIONARY operand (32 cols -> ~27ns ldweights) and ttT as
           the moving operand (128 b-cols), accumulating into
           uT_ps[32(j%2):.., j//2, :] -- 4 tiles packed [64p, 2slot]
           (partition base 96 is not encodable: PE quadrant-3 bug).
           (v2 used ttT as the stationary: a 128-col weight load for 32
           moving cols, making the scatter LDWEIGHTS-bound: +43us/pass.)
  Per group of 4 tiles (1024 rows):
    - ACT: evacuate uT_ps [64, 2, 128] -> SBUF with per-partition scale
           1/cap[l] (uT partition is 32j''+16rr+l, so scale[p]=capinv[p%16])
    - PE : two f32 [64,128] transposes -> u2_ps [128 b, 128 (s,j'',rr,l)]
    - ACT: Square+accum_out -> accq (sum u^2), Square(s8)+accum -> accs2,
           Copy(m8)+accum -> accm
    - DVE: reduce_sum / reduce_max over l -> s8, m8
  Host: tiny final reduction across cores.
"""

from contextlib import ExitStack

import numpy as np

import concourse.bass as bass
import concourse.tile as tile
from concourse import mybir
from concourse.bass_utils import run_bass_kernel_spmd
from bass_rust import ScopedClock

N_CORES = 8
B, T, D, L = 65536, 792, 99, 16
ROWS = B // N_CORES  # 8192 rows per core
P = 128
RR = 2  # rows per partition
TROWS = P * RR  # 256 rows per tile
NT = ROWS // TROWS  # 32 tiles per core
TW = RR * T  # 1584 free elems per tile
NCH = (TW + P - 1) // P  # 13 chunks
# chunk c covers tt columns CH_OFF[c] .. CH_OFF[c]+127; the last chunk
# overlaps chunk 11 by 80 columns (mask rows for the overlap are zero)
# so every transpose/scatter is a uniform full-width 128 chunk.
CH_OFF = [c * P for c in range(NCH - 1)] + [TW - P]
NCA, NCB = 8, 5  # chunks in PSUM bank A / bank B
G = 4  # tiles per stats group (G tiles -> one uT PSUM tile)
NG = NT // G  # 8 groups
GROWS = G * TROWS  # 1024 rows per group
M = RR * L  # 32 uT partitions per tile

F32 = mybir.dt.float32
BF16 = mybir.dt.bfloat16
X = mybir.AxisListType
AF = mybir.ActivationFunctionType


class _TileContext(tile.TileContext):
    """Workaround: this walrus build allows only 1 sync-wait per
    instruction; stock TileContext packs one wait per outstanding proc
    onto the single tail drain. Spread them across multiple drains."""

    def _drain_and_barrier(self, tick_clock, wait_clock):
        nc = self.nc
        drain_inst = nc.sync.drain()
        wait_clock.add_sem_waits(
            drain_inst.ins, ScopedClock({None: tick_clock.global_clock})
        )
        si = drain_inst.ins.sync_info
        waits = list(si.on_wait) if si is not None and si.on_wait else []
        if len(waits) > 1:
            drain_inst.ins.sync_info = mybir.SyncInfo(
                on_wait=[waits[0]], on_update=list(si.on_update or [])
            )
            for w in waits[1:]:
                d = nc.sync.drain()
                d.ins.sync_info = mybir.SyncInfo(on_wait=[w], on_update=[])
        nc.all_engine_barrier()
        assert self.sems is not None
        popped = nc._tile_sem_poison_stack.pop()
        assert popped is self._sem_poison
        nc.clear_and_free_semaphores(list(self.sems.allocated().values()))
        nc.all_engine_barrier()


def _split_multi_waits(nc):
    """This walrus build accepts only 1 sync-wait per instruction (2 for
    EventSemaphore). Hoist extra semaphore waits onto same-engine NOPs
    inserted immediately before the instruction (engine queues are strict
    FIFO, so a preceding wait-NOP is semantically identical)."""
    for fn in nc.m.functions:
        for blk in fn.blocks:
            insts = blk.instructions
            out = []
            for inst in insts:
                si = inst.sync_info
                waits = list(si.on_wait) if si is not None and si.on_wait else []
                cap = 2 if isinstance(inst, mybir.InstEventSemaphore) else 1
                if len(waits) > cap and inst.engine != mybir.EngineType.Unassigned:
                    for w in waits[:-1]:
                        nop = mybir.InstNoOp(
                            name=f"{inst.name}-w{len(out)}",
                            engine=inst.engine,
                            sync_info=mybir.SyncInfo(on_wait=[w], on_update=[]),
                            bass_nofuse=True,
                        )
                        nc.register_instruction(nop, overwrite=True)
                        out.append(nop)
                    inst.sync_info = mybir.SyncInfo(
                        on_wait=[waits[-1]], on_update=list(si.on_update or [])
                    )
                out.append(inst)
            blk.instructions = out


def build_kernel(
    repeat=1,
    n_tiles=NT,
    stages=("dma", "mul", "trans", "scat", "stats"),
    loop=None,
    io_bufs=5,
    spread=(1, 2, 3),
):
    """Build the per-core Bass module. loop: wrap the pass in a hardware
    For_i loop executing it `loop` times on-device (timing builds).
    stages: knock out pipeline stages for profiling."""
    ng = max(1, n_tiles // G)
    nc = bass.Bass("TRN2", target_bir_lowering=False, debug=False, num_devices=1)
    pred_d = nc.dram_tensor("pred", [ROWS, T], F32, kind="ExternalInput")
    dem_d = nc.dram_tensor("dem", [ROWS, D], F32, kind="ExternalInput")
    mask_d = nc.dram_tensor("mask", [P, NCH * M], BF16, kind="ExternalInput")
    ident_d = nc.dram_tensor("ident", [P, P], BF16, kind="ExternalInput")
    identf_d = nc.dram_tensor("identf", [P, P], F32, kind="ExternalInput")
    capinv_d = nc.dram_tensor("capinv", [P, 1], F32, kind="ExternalInput")
    out_d = nc.dram_tensor("partials", [3, P, ng], F32, kind="ExternalOutput")

    with _TileContext(nc) as tc:
        with ExitStack() as ctx:
            singles = ctx.enter_context(tc.tile_pool(name="singles", bufs=1))
            io = ctx.enter_context(tc.tile_pool(name="io", bufs=io_bufs))
            work = ctx.enter_context(tc.tile_pool(name="work", bufs=3))
            evac = ctx.enter_context(tc.tile_pool(name="evac", bufs=3))
            usb = ctx.enter_context(tc.tile_pool(name="usb", bufs=2))
            small = ctx.enter_context(tc.tile_pool(name="small", bufs=2))
            tpsA = ctx.enter_context(tc.tile_pool(name="tpsA", bufs=2, space="PSUM"))
            tpsB = ctx.enter_context(tc.tile_pool(name="tpsB", bufs=2, space="PSUM"))
            uTp = ctx.enter_context(tc.tile_pool(name="uTp", bufs=2, space="PSUM"))
            u2p = ctx.enter_context(tc.tile_pool(name="u2p", bufs=2, space="PSUM"))

            ident_t = singles.tile([P, P], BF16)
            nc.sync.dma_start(ident_t[:], ident_d.ap())
            identf_t = singles.tile([P, P], F32)
            nc.sync.dma_start(identf_t[:], identf_d.ap())
            mask_t = singles.tile([P, NCH, M], BF16)
            nc.sync.dma_start(
                mask_t[:], mask_d.ap().rearrange("p (c m) -> p c m", c=NCH)
            )
            capinv_t = singles.tile([P, 1], F32)
            nc.sync.dma_start(capinv_t[:], capinv_d.ap())
            accq = singles.tile([P, ng], F32)
            accs2 = singles.tile([P, ng], F32)
            accm = singles.tile([P, ng], F32)
            for acc in (accq, accs2, accm):
                nc.gpsimd.memset(acc[:], 0.0)

            def make_stats(uT_ps, g):
                """Three stats-chain segments for group g, emitted at
                j=1/2/3 of group g+1 so each cross-engine hop hides behind
                a full tile of independent work in the strict-FIFO queues.
                accq comes straight off uT_ps (Square of the scaled value,
                free-dim accumulated); the bf16 transpose of u feeds only
                the l-reductions (sum for accs2, max for accm) -- bf16
                rounding of u shifts the final loss by ~1e-5 relative."""
                state = {}

                def emit1():  # ACT: accq + u_sb(bf16)
                    usq = small.tile([2 * M, 2, P], F32)
                    nc.scalar.activation(
                        out=usq[:],
                        in_=uT_ps[:],
                        func=AF.Square,
                        scale=capinv_t[0 : 2 * M, 0:1],
                        accum_out=accq[0 : 2 * M, g : g + 1],
                    )
                    u_sb = usb.tile([2 * M, 2, P], BF16)
                    nc.scalar.activation(
                        out=u_sb[:],
                        in_=uT_ps[:],
                        func=AF.Copy,
                        scale=capinv_t[0 : 2 * M, 0:1],
                    )
                    state["u_sb"] = u_sb

                def emit2():  # PE: bf16 transposes back to [b, (s,j,rr,l)]
                    u2_ps = u2p.tile([P, 2, 2 * M], BF16)
                    for s in range(2):
                        nc.tensor.transpose(
                            out=u2_ps[:, s, :],
                            in_=state["u_sb"][:, s, :],
                            identity=ident_t[0 : 2 * M, 0 : 2 * M],
                        )
                    state["u2"] = u2_ps

                def emit3():  # DVE reduces + ACT accumulations
                    u2v = state["u2"][:].rearrange("p s (m l) -> p (s m) l", l=L)
                    s8 = small.tile([P, G * RR], F32)
                    nc.vector.reduce_sum(out=s8[:], in_=u2v, axis=X.X)
                    m8 = small.tile([P, G * RR], F32)
                    nc.vector.reduce_max(out=m8[:], in_=u2v, axis=X.X)
                    sq8 = small.tile([P, G * RR], F32)
                    nc.scalar.activation(
                        out=sq8[:],
                        in_=s8[:],
                        func=AF.Square,
                        accum_out=accs2[:, g : g + 1],
                    )
                    md = small.tile([P, G * RR], F32)
                    nc.scalar.activation(
                        out=md[:],
                        in_=m8[:],
                        func=AF.Copy,
                        accum_out=accm[:, g : g + 1],
                    )

                return [emit1, emit2, emit3]

            loop_cm = tc.For_i(0, loop, 1) if loop is not None else None
            if loop_cm is not None:
                loop_cm.__enter__()
            pending_stats = []
            for rep in range(repeat):
                for g in range(ng):
                    gi = g % (n_tiles // G)
                    pred_g = io.tile([P, G, TW], F32)
                    dem_g = io.tile([P, G, RR * D], F32)
                    if "dma" in stages:
                        nc.sync.dma_start(
                            pred_g[:],
                            pred_d.ap()[gi * GROWS : (gi + 1) * GROWS, :].rearrange(
                                "(r p rr) t -> p r (rr t)", p=P, rr=RR
                            ),
                        )
                        nc.sync.dma_start(
                            dem_g[:],
                            dem_d.ap()[gi * GROWS : (gi + 1) * GROWS, :].rearrange(
                                "(r p rr) d -> p r (rr d)", p=P, rr=RR
                            ),
                        )
                    uT_ps = uTp.tile([2 * M, 2, P], F32)
                    for j in range(G):
                        if "mul" not in stages:
                            continue
                        tt = work.tile([P, TW], BF16)
                        nc.vector.tensor_tensor(
                            out=tt[:].rearrange("p (e k) -> p e k", k=8),
                            in0=pred_g[:, j, :].rearrange("p (e k) -> p e k", k=8),
                            in1=dem_g[:, j, :]
                            .unsqueeze(2)
                            .broadcast_to([P, RR * D, 8]),
                            op=mybir.AluOpType.mult,
                        )
                        if "trans" not in stages:
                            continue
                        ttA_ps = tpsA.tile([P, NCA, P], BF16)
                        ttB_ps = tpsB.tile([P, NCB, P], BF16)
                        for c in range(NCH):
                            dst = (
                                ttA_ps[:, c, :] if c < NCA else ttB_ps[:, c - NCA, :]
                            )
                            nc.tensor.transpose(
                                out=dst,
                                in_=tt[:, CH_OFF[c] : CH_OFF[c] + P],
                                identity=ident_t[:],
                            )
                        ttA = evac.tile([P, NCA, P], BF16)
                        nc.scalar.copy(out=ttA[:], in_=ttA_ps[:])
                        ttB = evac.tile([P, NCB, P], BF16)
                        nc.scalar.copy(out=ttB[:], in_=ttB_ps[:])
                        if "scat" not in stages:
                            continue
                        jp, js = j % 2, j // 2
                        for c in range(NCH):
                            src = ttA[:, c, :] if c < NCA else ttB[:, c - NCA, :]
                            nc.tensor.matmul(
                                out=uT_ps[M * jp : M * (jp + 1), js, :],
                                lhsT=mask_t[:, c, :],
                                rhs=src,
                                start=(c == 0),
                                stop=(c == NCH - 1),
                            )
                        if pending_stats and j in spread:
                            pending_stats.pop(0)()
                        while pending_stats and j == G - 1:
                            pending_stats.pop(0)()
                    # --- stats for this group of G tiles: segments run at
                    # j=1/2/3 of the next group (see make_stats docstring) ---
                    if "stats" not in stages:
                        continue
                    assert not pending_stats
                    pending_stats = make_stats(uT_ps, g)
            for f in pending_stats:
                f()
            pending_stats = []
            if loop_cm is not None:
                loop_cm.__exit__(None, None, None)
            nc.sync.dma_start(out_d.ap()[0], accq[:])
            nc.sync.dma_start(out_d.ap()[1], accs2[:])
            nc.sync.dma_start(out_d.ap()[2], accm[:])
    _split_multi_waits(nc)
    return nc


def make_constants(tunnel_to_link, link_capacities):
    t2l = np.asarray(tunnel_to_link).astype(np.int64).ravel()
    cap = np.asarray(link_capacities, dtype=np.float32).ravel()
    # mask[k, c, rr*L + l]: chunk c covers free idx f = CH_OFF[c] + k,
    # f = rr*T + t; one-hot into (rr, link(t)).  The last chunk overlaps
    # chunk NCH-2 by 80 columns; overlap rows stay zero so those tunnels
    # are only counted once.
    mask = np.zeros((P, NCH, M), dtype=np.float32)
    for c in range(NCH):
        k_lo = 0 if c < NCH - 1 else (NCH - 1) * P - CH_OFF[c]
        for k in range(k_lo, P):
            f = CH_OFF[c] + k
            rr, t = divmod(f, T)
            mask[k, c, rr * L + int(t2l[t])] = 1.0
    mask = mask.reshape(P, NCH * M)
    ident = np.eye(P, dtype=np.float32)
    # uT partition p = 32j + 16rr + l  ->  l = p % 16
    capinv = (1.0 / (cap + 1e-8)).astype(np.float32)
    capinv_col = capinv[np.arange(P) % L][:, None].copy()
    return mask, ident, ident.copy(), capinv_col


def _to_bf16(a):
    # numpy has no bf16; round-to-nearest-even via ml_dtypes if present,
    # else truncate+round manually and keep uint16 view.
    try:
        import ml_dtypes

        return a.astype(ml_dtypes.bfloat16)
    except ImportError:
        x = a.astype(np.float32).view(np.uint32)
        x = (x + 0x7FFF + ((x >> 16) & 1)) >> 16
        return x.astype(np.uint16)


def run_cores(nc, pred, dem, mask, ident, identf, capinv, **kw):
    pred = np.ascontiguousarray(np.asarray(pred, dtype=np.float32))
    dem = np.ascontiguousarray(np.asarray(dem, dtype=np.float32))
    mask_bf = _to_bf16(mask)
    ident_bf = _to_bf16(ident)
    identf = np.asarray(identf, dtype=np.float32)
    capinv = np.asarray(capinv, dtype=np.float32)
    in_maps = []
    for i in range(N_CORES):
        in_maps.append(
            {
                "pred": pred[i * ROWS : (i + 1) * ROWS],
                "dem": dem[i * ROWS : (i + 1) * ROWS],
                "mask": mask_bf,
                "ident": ident_bf,
                "identf": identf,
                "capinv": capinv,
            }
        )
    return run_bass_kernel_spmd(nc, in_maps, core_ids=list(range(N_CORES)), **kw)


def combine_partials(partials_list):
    q = s2 = m = 0.0
    for p in partials_list:
        p = np.asarray(p, dtype=np.float64)
        q += p[0].sum()
        s2 += p[1].sum()
        m += p[2].sum()
    var_mean = (q - s2 / L) / (L - 1) / B
    return var_mean + 0.5 * m / B


def kernel(pred_ratios, demands, tunnel_to_link, link_capacities):
    mask, ident, identf, capinv = make_constants(tunnel_to_link, link_capacities)
    nc = build_kernel()
    res = run_cores(nc, pred_ratios, demands, mask, ident, identf, capinv)
    loss = combine_partials([r["partials"] for r in res.results])
    return np.array(loss, dtype=np.float32)


# revision 38
# speedup vs baseline: 1.0278x; 1.0278x over previous
"""Trainium2 Bass kernel for nn_LocalLoadBalancingLoss (v3).

loss = mean_b var_l(u) + 0.5 * mean_b max_l(u),
u[b,l] = (sum_{t: link(t)=l} pred[b,t] * dem[b, t//8]) / (cap[l] + 1e-8)

Strategy (pure data parallel over batch, 8 cores x 8192 rows):
  Row-paired tiles: one tile = 256 rows laid out [128p, 2rr, ...] with
  partition p holding DRAM rows {base+2p, base+2p+1}; group DMAs move
  ~3.2 MB of contiguous >=6 KB descriptors per dma_start.

  Per 256-row tile (free width TW = 2*792 = 1584, padded to 1664):
    - DVE: tt(bf16) = pred * broadcast(dem)   (two ops, one per rr)
    - PE : 13x transpose of 128-wide tt chunks -> PSUM (bf16)
    - ACT: evacuate ttT PSUM -> SBUF (2 copies: 8-chunk + 5-chunk banks)
    - PE : 13x scatter matmul with the one-hot link mask as the
           STATIONARY operand (32 cols -> ~27ns ldweights) and ttT as
           the moving operand (128 b-cols), accumulating into
           uT_ps[32(j%2):.., j//2, :] -- 4 tiles packed [64p, 2slot]
           (partition base 96 is not encodable: PE quadrant-3 bug).
           (v2 used ttT as the stationary: a 128-col weight load for 32
           moving cols, making the scatter LDWEIGHTS-bound: +43us/pass.)
  Per group of 4 tiles (1024 rows):
    - ACT: evacuate uT_ps [64, 2, 128] -> SBUF with per-partition scale
           1/cap[l] (uT partition is 32j''+16rr+l, so scale[p]=capinv[p%16])
    - PE : two f32 [64,128] transposes -> u2_ps [128 b, 128 (s,j'',rr,l)]
    - ACT: Square+accum_out -> accq (sum u^2), Square(s8)+accum -> accs2,
           Copy(m8)+accum -> accm
    - DVE: reduce_sum / reduce_max over l -> s8, m8
  Host: tiny final reduction across cores.
"""

from contextlib import ExitStack

import numpy as np

import concourse.bass as bass
import concourse.tile as tile
from concourse import mybir
from concourse.bass_utils import run_bass_kernel_spmd
from bass_rust import ScopedClock

N_CORES = 8
B, T, D, L = 65536, 792, 99, 16
ROWS = B // N_CORES  # 8192 rows per core
P = 128
RR = 2  # rows per partition
TROWS = P * RR  # 256 rows per tile
NT = ROWS // TROWS  # 32 tiles per core
TW = RR * T  # 1584 free elems per tile
NCH = (TW + P - 1) // P  # 13 chunks
# chunk c covers tt columns CH_OFF[c] .. CH_OFF[c]+127; the last chunk
# overlaps chunk 11 by 80 columns (mask rows for the overlap are zero)
# so every transpose/scatter is a uniform full-width 128 chunk.
CH_OFF = [c * P for c in range(NCH - 1)] + [TW - P]
NCA, NCB = 8, 5  # chunks in PSUM bank A / bank B
G = 4  # tiles per stats group (G tiles -> one uT PSUM tile)
NG = NT // G  # 8 groups
GROWS = G * TROWS  # 1024 rows per group
M = RR * L  # 32 uT partitions per tile

F32 = mybir.dt.float32
BF16 = mybir.dt.bfloat16
FP8 = mybir.dt.float8e4
X = mybir.AxisListType
AF = mybir.ActivationFunctionType


class _TileContext(tile.TileContext):
    """Workaround: this walrus build allows only 1 sync-wait per
    instruction; stock TileContext packs one wait per outstanding proc
    onto the single tail drain. Spread them across multiple drains."""

    def _drain_and_barrier(self, tick_clock, wait_clock):
        nc = self.nc
        drain_inst = nc.sync.drain()
        wait_clock.add_sem_waits(
            drain_inst.ins, ScopedClock({None: tick_clock.global_clock})
        )
        si = drain_inst.ins.sync_info
        waits = list(si.on_wait) if si is not None and si.on_wait else []
        if len(waits) > 1:
            drain_inst.ins.sync_info = mybir.SyncInfo(
                on_wait=[waits[0]], on_update=list(si.on_update or [])
            )
            for w in waits[1:]:
                d = nc.sync.drain()
                d.ins.sync_info = mybir.SyncInfo(on_wait=[w], on_update=[])
        nc.all_engine_barrier()
        assert self.sems is not None
        popped = nc._tile_sem_poison_stack.pop()
        assert popped is self._sem_poison
        nc.clear_and_free_semaphores(list(self.sems.allocated().values()))
        nc.all_engine_barrier()


def _split_multi_waits(nc):
    """This walrus build accepts only 1 sync-wait per instruction (2 for
    EventSemaphore). Hoist extra semaphore waits onto same-engine NOPs
    inserted immediately before the instruction (engine queues are strict
    FIFO, so a preceding wait-NOP is semantically identical)."""
    for fn in nc.m.functions:
        for blk in fn.blocks:
            insts = blk.instructions
            out = []
            for inst in insts:
                si = inst.sync_info
                waits = list(si.on_wait) if si is not None and si.on_wait else []
                cap = 2 if isinstance(inst, mybir.InstEventSemaphore) else 1
                if len(waits) > cap and inst.engine != mybir.EngineType.Unassigned:
                    for w in waits[:-1]:
                        nop = mybir.InstNoOp(
                            name=f"{inst.name}-w{len(out)}",
                            engine=inst.engine,
                            sync_info=mybir.SyncInfo(on_wait=[w], on_update=[]),
                            bass_nofuse=True,
                        )
                        nc.register_instruction(nop, overwrite=True)
                        out.append(nop)
                    inst.sync_info = mybir.SyncInfo(
                        on_wait=[waits[-1]], on_update=list(si.on_update or [])
                    )
                out.append(inst)
            blk.instructions = out


def build_kernel(
    repeat=1,
    n_tiles=NT,
    stages=("dma", "mul", "trans", "scat", "stats"),
    loop=None,
    io_bufs=5,
    spread=(1, 2, 3),
    work_bufs=3,
    evac_bufs=3,
    tps_bufs=2,
    dem_q="sync",
    tt_dtype="fp8",
):
    """Build the per-core Bass module. loop: wrap the pass in a hardware
    For_i loop executing it `loop` times on-device (timing builds).
    stages: knock out pipeline stages for profiling."""
    ng = max(1, n_tiles // G)
    TDT = FP8 if tt_dtype == "fp8" else BF16
    nc = bass.Bass("TRN2", target_bir_lowering=False, debug=False, num_devices=1)
    pred_d = nc.dram_tensor("pred", [ROWS, T], F32, kind="ExternalInput")
    dem_d = nc.dram_tensor("dem", [ROWS, D], F32, kind="ExternalInput")
    mask_d = nc.dram_tensor("mask", [P, NCH * M], TDT, kind="ExternalInput")
    ident_d = nc.dram_tensor("ident", [P, P], TDT, kind="ExternalInput")
    identst_d = nc.dram_tensor("identst", [P, P], BF16, kind="ExternalInput")
    capinv_d = nc.dram_tensor("capinv", [P, 1], F32, kind="ExternalInput")
    out_d = nc.dram_tensor("partials", [3, P, ng], F32, kind="ExternalOutput")

    with _TileContext(nc) as tc:
        with ExitStack() as ctx:
            singles = ctx.enter_context(tc.tile_pool(name="singles", bufs=1))
            io = ctx.enter_context(tc.tile_pool(name="io", bufs=io_bufs))
            work = ctx.enter_context(tc.tile_pool(name="work", bufs=work_bufs))
            evac = ctx.enter_context(tc.tile_pool(name="evac", bufs=evac_bufs))
            usb = ctx.enter_context(tc.tile_pool(name="usb", bufs=2))
            small = ctx.enter_context(tc.tile_pool(name="small", bufs=2))
            tpsA = ctx.enter_context(
                tc.tile_pool(name="tpsA", bufs=tps_bufs, space="PSUM")
            )
            tpsB = ctx.enter_context(
                tc.tile_pool(name="tpsB", bufs=tps_bufs, space="PSUM")
            )
            uTp = ctx.enter_context(tc.tile_pool(name="uTp", bufs=2, space="PSUM"))
            u2p = ctx.enter_context(tc.tile_pool(name="u2p", bufs=2, space="PSUM"))

            ident_t = singles.tile([P, P], TDT)
            nc.sync.dma_start(ident_t[:], ident_d.ap())
            identst_t = singles.tile([P, P], BF16)
            nc.sync.dma_start(identst_t[:], identst_d.ap())
            mask_t = singles.tile([P, NCH, M], TDT)
            nc.sync.dma_start(
                mask_t[:], mask_d.ap().rearrange("p (c m) -> p c m", c=NCH)
            )
            capinv_t = singles.tile([P, 1], F32)
            nc.sync.dma_start(capinv_t[:], capinv_d.ap())
            accq = singles.tile([P, ng], F32)
            accs2 = singles.tile([P, ng], F32)
            accm = singles.tile([P, ng], F32)
            for acc in (accq, accs2, accm):
                nc.gpsimd.memset(acc[:], 0.0)

            def make_stats(uT_ps, g):
                """Three stats-chain segments for group g, emitted at
                j=1/2/3 of group g+1 so each cross-engine hop hides behind
                a full tile of independent work in the strict-FIFO queues.
                accq comes straight off uT_ps (Square of the scaled value,
                free-dim accumulated); the bf16 transpose of u feeds only
                the l-reductions (sum for accs2, max for accm) -- bf16
                rounding of u shifts the final loss by ~1e-5 relative."""
                state = {}

                def emit1():  # ACT: accq + u_sb(bf16)
                    usq = small.tile([2 * M, 2, P], F32)
                    nc.scalar.activation(
                        out=usq[:],
                        in_=uT_ps[:],
                        func=AF.Square,
                        scale=capinv_t[0 : 2 * M, 0:1],
                        accum_out=accq[0 : 2 * M, g : g + 1],
                    )
                    u_sb = usb.tile([2 * M, 2, P], BF16)
                    nc.scalar.activation(
                        out=u_sb[:],
                        in_=uT_ps[:],
                        func=AF.Copy,
                        scale=capinv_t[0 : 2 * M, 0:1],
                    )
                    state["u_sb"] = u_sb

                def emit2():  # PE: bf16 transposes back to [b, (s,j,rr,l)]
                    u2_ps = u2p.tile([P, 2, 2 * M], BF16)
                    for s in range(2):
                        nc.tensor.transpose(
                            out=u2_ps[:, s, :],
                            in_=state["u_sb"][:, s, :],
                            identity=identst_t[0 : 2 * M, 0 : 2 * M],
                        )
                    state["u2"] = u2_ps

                def emit3():  # DVE reduces + ACT accumulations
                    u2v = state["u2"][:].rearrange("p s (m l) -> p (s m) l", l=L)
                    s8 = small.tile([P, G * RR], F32)
                    nc.vector.reduce_sum(out=s8[:], in_=u2v, axis=X.X)
                    m8 = small.tile([P, G * RR], F32)
                    nc.vector.reduce_max(out=m8[:], in_=u2v, axis=X.X)
                    sq8 = small.tile([P, G * RR], F32)
                    nc.scalar.activation(
                        out=sq8[:],
                        in_=s8[:],
                        func=AF.Square,
                        accum_out=accs2[:, g : g + 1],
                    )
                    md = small.tile([P, G * RR], F32)
                    nc.scalar.activation(
                        out=md[:],
                        in_=m8[:],
                        func=AF.Copy,
                        accum_out=accm[:, g : g + 1],
                    )

                return [emit1, emit2, emit3]

            loop_cm = tc.For_i(0, loop, 1) if loop is not None else None
            if loop_cm is not None:
                loop_cm.__enter__()
            pending_stats = []
            for rep in range(repeat):
                for g in range(ng):
                    gi = g % (n_tiles // G)
                    pred_g = io.tile([P, G, TW], F32)
                    dem_g = io.tile([P, G, RR * D], F32)
                    if "dma" in stages:
                        nc.sync.dma_start(
                            pred_g[:],
                            pred_d.ap()[gi * GROWS : (gi + 1) * GROWS, :].rearrange(
                                "(r p rr) t -> p r (rr t)", p=P, rr=RR
                            ),
                        )
                        dem_eng = nc.sync if dem_q == "sync" else nc.scalar
                        dem_eng.dma_start(
                            dem_g[:],
                            dem_d.ap()[gi * GROWS : (gi + 1) * GROWS, :].rearrange(
                                "(r p rr) d -> p r (rr d)", p=P, rr=RR
                            ),
                        )
                    uT_ps = uTp.tile([2 * M, 2, P], F32)
                    for j in range(G):
                        if "mul" not in stages:
                            continue
                        tt = work.tile([P, TW], TDT)
                        nc.vector.tensor_tensor(
                            out=tt[:].rearrange("p (e k) -> p e k", k=8),
                            in0=pred_g[:, j, :].rearrange("p (e k) -> p e k", k=8),
                            in1=dem_g[:, j, :]
                            .unsqueeze(2)
                            .broadcast_to([P, RR * D, 8]),
                            op=mybir.AluOpType.mult,
                        )
                        if "trans" not in stages:
                            continue
                        # fp8 transpose mode writes with element step 2, so
                        # the PSUM chunks keep a [.., P, 2] layout (same bank
                        # footprint as bf16); the ACT evacuation repacks.
                        es = 2 if tt_dtype == "fp8" else 1
                        ttA_ps = tpsA.tile([P, NCA, P, es], TDT)
                        ttB_ps = tpsB.tile([P, NCB, P, es], TDT)
                        for c in range(NCH):
                            dst = (
                                ttA_ps[:, c, :, 0]
                                if c < NCA
                                else ttB_ps[:, c - NCA, :, 0]
                            )
                            nc.tensor.transpose(
                                out=dst,
                                in_=tt[:, CH_OFF[c] : CH_OFF[c] + P],
                                identity=ident_t[:],
                            )
                        ttA = evac.tile([P, NCA, P], TDT)
                        nc.scalar.copy(out=ttA[:], in_=ttA_ps[:, :, :, 0])
                        ttB = evac.tile([P, NCB, P], TDT)
                        nc.scalar.copy(out=ttB[:], in_=ttB_ps[:, :, :, 0])
                        if "scat" not in stages:
                            continue
                        jp, js = j % 2, j // 2
                        for c in range(NCH):
                            src = ttA[:, c, :] if c < NCA else ttB[:, c - NCA, :]
                            nc.tensor.matmul(
                                out=uT_ps[M * jp : M * (jp + 1), js, :],
                                lhsT=mask_t[:, c, :],
                                rhs=src,
                                start=(c == 0),
                                stop=(c == NCH - 1),
                            )
                        for _ in range(spread.count(j)):
                            if pending_stats:
                                pending_stats.pop(0)()
                        while pending_stats and j == G - 1:
                            pending_stats.pop(0)()
                    # --- stats for this group of G tiles: segments run at
                    # j=1/2/3 of the next group (see make_stats docstring) ---
                    if "stats" not in stages:
                        continue
                    assert not pending_stats
                    pending_stats = make_stats(uT_ps, g)
            for f in pending_stats:
                f()
            pending_stats = []
            if loop_cm is not None:
                loop_cm.__exit__(None, None, None)
            nc.sync.dma_start(out_d.ap()[0], accq[:])
            nc.sync.dma_start(out_d.ap()[1], accs2[:])
            nc.sync.dma_start(out_d.ap()[2], accm[:])
    _split_multi_waits(nc)
    return nc


def make_constants(tunnel_to_link, link_capacities):
    t2l = np.asarray(tunnel_to_link).astype(np.int64).ravel()
    cap = np.asarray(link_capacities, dtype=np.float32).ravel()
    # mask[k, c, rr*L + l]: chunk c covers free idx f = CH_OFF[c] + k,
    # f = rr*T + t; one-hot into (rr, link(t)).  The last chunk overlaps
    # chunk NCH-2 by 80 columns; overlap rows stay zero so those tunnels
    # are only counted once.
    mask = np.zeros((P, NCH, M), dtype=np.float32)
    for c in range(NCH):
        k_lo = 0 if c < NCH - 1 else (NCH - 1) * P - CH_OFF[c]
        for k in range(k_lo, P):
            f = CH_OFF[c] + k
            rr, t = divmod(f, T)
            mask[k, c, rr * L + int(t2l[t])] = 1.0
    mask = mask.reshape(P, NCH * M)
    ident = np.eye(P, dtype=np.float32)
    # uT partition p = 32j'' + 16rr + l  ->  l = p % 16
    capinv = (1.0 / (cap + 1e-8)).astype(np.float32)
    capinv_col = capinv[np.arange(P) % L][:, None].copy()
    return mask, ident, capinv_col


def _to_bf16(a):
    import ml_dtypes

    return a.astype(ml_dtypes.bfloat16)


def _to_fp8(a):
    import ml_dtypes

    return a.astype(ml_dtypes.float8_e4m3fn)


def run_cores(nc, pred, dem, mask, ident, capinv, tt_dtype="fp8", **kw):
    pred = np.ascontiguousarray(np.asarray(pred, dtype=np.float32))
    dem = np.ascontiguousarray(np.asarray(dem, dtype=np.float32))
    conv = _to_fp8 if tt_dtype == "fp8" else _to_bf16
    mask_c = conv(mask)
    ident_c = conv(ident)
    identst = _to_bf16(ident)
    capinv = np.asarray(capinv, dtype=np.float32)
    in_maps = []
    for i in range(N_CORES):
        in_maps.append(
            {
                "pred": pred[i * ROWS : (i + 1) * ROWS],
                "dem": dem[i * ROWS : (i + 1) * ROWS],
                "mask": mask_c,
                "ident": ident_c,
                "identst": identst,
                "capinv": capinv,
            }
        )
    return run_bass_kernel_spmd(nc, in_maps, core_ids=list(range(N_CORES)), **kw)


def combine_partials(partials_list):
    q = s2 = m = 0.0
    for p in partials_list:
        p = np.asarray(p, dtype=np.float64)
        q += p[0].sum()
        s2 += p[1].sum()
        m += p[2].sum()
    var_mean = (q - s2 / L) / (L - 1) / B
    return var_mean + 0.5 * m / B


def kernel(pred_ratios, demands, tunnel_to_link, link_capacities):
    mask, ident, capinv = make_constants(tunnel_to_link, link_capacities)
    nc = build_kernel()
    res = run_cores(nc, pred_ratios, demands, mask, ident, capinv)
    loss = combine_partials([r["partials"] for r in res.results])
    return np.array(loss, dtype=np.float32)


# revision 48
# speedup vs baseline: 1.2345x; 1.2010x over previous
"""Trainium2 Bass kernel for nn_LocalLoadBalancingLoss (v3).

loss = mean_b var_l(u) + 0.5 * mean_b max_l(u),
u[b,l] = (sum_{t: link(t)=l} pred[b,t] * dem[b, t//8]) / (cap[l] + 1e-8)

Strategy (pure data parallel over batch, 8 cores x 8192 rows):
  Row-paired tiles: one tile = 256 rows laid out [128p, 2rr, ...] with
  partition p holding DRAM rows {base+2p, base+2p+1}; group DMAs move
  ~3.2 MB of contiguous >=6 KB descriptors per dma_start.

  Per 256-row tile (free width TW = 2*792 = 1584, padded to 1664):
    - DVE: tt(bf16) = pred * broadcast(dem)   (two ops, one per rr)
    - PE : 13x transpose of 128-wide tt chunks -> PSUM (bf16)
    - ACT: evacuate ttT PSUM -> SBUF (2 copies: 8-chunk + 5-chunk banks)
    - PE : 13x scatter matmul with the one-hot link mask as the
           STATIONARY operand (32 cols -> ~27ns ldweights) and ttT as
           the moving operand (128 b-cols), accumulating into
           uT_ps[32(j%2):.., j//2, :] -- 4 tiles packed [64p, 2slot]
           (partition base 96 is not encodable: PE quadrant-3 bug).
           (v2 used ttT as the stationary: a 128-col weight load for 32
           moving cols, making the scatter LDWEIGHTS-bound: +43us/pass.)
  Per group of 4 tiles (1024 rows):
    - ACT: evacuate uT_ps [64, 2, 128] -> SBUF with per-partition scale
           1/cap[l] (uT partition is 32j''+16rr+l, so scale[p]=capinv[p%16])
    - PE : two f32 [64,128] transposes -> u2_ps [128 b, 128 (s,j'',rr,l)]
    - ACT: Square+accum_out -> accq (sum u^2), Square(s8)+accum -> accs2,
           Copy(m8)+accum -> accm
    - DVE: reduce_sum / reduce_max over l -> s8, m8
  Host: tiny final reduction across cores.
"""

from contextlib import ExitStack

import numpy as np

import concourse.bass as bass
import concourse.tile as tile
from concourse import mybir
from concourse.bass_utils import run_bass_kernel_spmd
from bass_rust import ScopedClock

N_CORES = 8
B, T, D, L = 65536, 792, 99, 16
ROWS = B // N_CORES  # 8192 rows per core
P = 128
RR = 2  # rows per partition
TROWS = P * RR  # 256 rows per tile
NT = ROWS // TROWS  # 32 tiles per core
TW = RR * T  # 1584 free elems per tile
NCH = (TW + P - 1) // P  # 13 chunks
# chunk c covers tt columns CH_OFF[c] .. CH_OFF[c]+127; the last chunk
# overlaps chunk 11 by 80 columns (mask rows for the overlap are zero)
# so every transpose/scatter is a uniform full-width 128 chunk.
CH_OFF = [c * P for c in range(NCH - 1)] + [TW - P]
NCA, NCB = 8, 5  # chunks in PSUM bank A / bank B
G = 4  # tiles per stats group (G tiles -> one uT PSUM tile)
NG = NT // G  # 8 groups
GROWS = G * TROWS  # 1024 rows per group
M = RR * L  # 32 uT partitions per tile

F32 = mybir.dt.float32
BF16 = mybir.dt.bfloat16
FP8 = mybir.dt.float8e4
X = mybir.AxisListType
AF = mybir.ActivationFunctionType


class _TileContext(tile.TileContext):
    """Workaround: this walrus build allows only 1 sync-wait per
    instruction; stock TileContext packs one wait per outstanding proc
    onto the single tail drain. Spread them across multiple drains."""

    def _drain_and_barrier(self, tick_clock, wait_clock):
        nc = self.nc
        drain_inst = nc.sync.drain()
        wait_clock.add_sem_waits(
            drain_inst.ins, ScopedClock({None: tick_clock.global_clock})
        )
        si = drain_inst.ins.sync_info
        waits = list(si.on_wait) if si is not None and si.on_wait else []
        if len(waits) > 1:
            drain_inst.ins.sync_info = mybir.SyncInfo(
                on_wait=[waits[0]], on_update=list(si.on_update or [])
            )
            for w in waits[1:]:
                d = nc.sync.drain()
                d.ins.sync_info = mybir.SyncInfo(on_wait=[w], on_update=[])
        nc.all_engine_barrier()
        assert self.sems is not None
        popped = nc._tile_sem_poison_stack.pop()
        assert popped is self._sem_poison
        nc.clear_and_free_semaphores(list(self.sems.allocated().values()))
        nc.all_engine_barrier()


def _split_multi_waits(nc):
    """This walrus build accepts only 1 sync-wait per instruction (2 for
    EventSemaphore). Hoist extra semaphore waits onto same-engine NOPs
    inserted immediately before the instruction (engine queues are strict
    FIFO, so a preceding wait-NOP is semantically identical)."""
    for fn in nc.m.functions:
        for blk in fn.blocks:
            insts = blk.instructions
            out = []
            for inst in insts:
                si = inst.sync_info
                waits = list(si.on_wait) if si is not None and si.on_wait else []
                cap = 2 if isinstance(inst, mybir.InstEventSemaphore) else 1
                if len(waits) > cap and inst.engine != mybir.EngineType.Unassigned:
                    for w in waits[:-1]:
                        nop = mybir.InstNoOp(
                            name=f"{inst.name}-w{len(out)}",
                            engine=inst.engine,
                            sync_info=mybir.SyncInfo(on_wait=[w], on_update=[]),
                            bass_nofuse=True,
                        )
                        nc.register_instruction(nop, overwrite=True)
                        out.append(nop)
                    inst.sync_info = mybir.SyncInfo(
                        on_wait=[waits[-1]], on_update=list(si.on_update or [])
                    )
                out.append(inst)
            blk.instructions = out


def build_kernel(
    repeat=1,
    n_tiles=NT,
    stages=("dma", "mul", "trans", "scat", "stats"),
    loop=None,
    io_bufs=5,
    spread=(1, 2, 3),
    work_bufs=3,
    evac_bufs=3,
    tps_bufs=2,
    dem_q="scalar",
    tt_dtype="bf16",
    pair_scat=False,
    pred_split=False,
    stagger=False,
    uT_bufs=2,
):
    """Build the per-core Bass module. loop: wrap the pass in a hardware
    For_i loop executing it `loop` times on-device (timing builds).
    stages: knock out pipeline stages for profiling."""
    ng = max(1, n_tiles // G)
    TDT = FP8 if tt_dtype == "fp8" else BF16
    nc = bass.Bass("TRN2", target_bir_lowering=False, debug=False, num_devices=1)
    pred_d = nc.dram_tensor("pred", [ROWS, T], F32, kind="ExternalInput")
    dem_d = nc.dram_tensor("dem", [ROWS, D], F32, kind="ExternalInput")
    mask_d = nc.dram_tensor("mask", [P, NCH * M], TDT, kind="ExternalInput")
    ident_d = nc.dram_tensor("ident", [P, P], TDT, kind="ExternalInput")
    identst_d = nc.dram_tensor("identst", [P, P], BF16, kind="ExternalInput")
    capinv_d = nc.dram_tensor("capinv", [P, 1], F32, kind="ExternalInput")
    out_d = nc.dram_tensor("partials", [3, P, ng], F32, kind="ExternalOutput")

    with _TileContext(nc) as tc:
        with ExitStack() as ctx:
            singles = ctx.enter_context(tc.tile_pool(name="singles", bufs=1))
            io = ctx.enter_context(tc.tile_pool(name="io", bufs=io_bufs))
            work = ctx.enter_context(tc.tile_pool(name="work", bufs=work_bufs))
            evac = ctx.enter_context(tc.tile_pool(name="evac", bufs=evac_bufs))
            usb = ctx.enter_context(tc.tile_pool(name="usb", bufs=2))
            small = ctx.enter_context(tc.tile_pool(name="small", bufs=2))
            tpsA = ctx.enter_context(
                tc.tile_pool(name="tpsA", bufs=tps_bufs, space="PSUM")
            )
            tpsB = ctx.enter_context(
                tc.tile_pool(
                    name="tpsB",
                    bufs=tps_bufs if uT_bufs == 2 else tps_bufs - 1,
                    space="PSUM",
                )
            )
            uTp = ctx.enter_context(
                tc.tile_pool(name="uTp", bufs=uT_bufs, space="PSUM")
            )
            u2p = ctx.enter_context(tc.tile_pool(name="u2p", bufs=2, space="PSUM"))

            ident_t = singles.tile([P, P], TDT)
            nc.sync.dma_start(ident_t[:], ident_d.ap())
            identst_t = singles.tile([P, P], BF16)
            nc.sync.dma_start(identst_t[:], identst_d.ap())
            mask_t = singles.tile([P, NCH, M], TDT)
            nc.sync.dma_start(
                mask_t[:], mask_d.ap().rearrange("p (c m) -> p c m", c=NCH)
            )
            capinv_t = singles.tile([P, 1], F32)
            nc.sync.dma_start(capinv_t[:], capinv_d.ap())
            accq = singles.tile([P, ng], F32)
            accs2 = singles.tile([P, ng], F32)
            accm = singles.tile([P, ng], F32)
            for acc in (accq, accs2, accm):
                nc.gpsimd.memset(acc[:], 0.0)

            def make_stats(uT_ps, g):
                """Three stats-chain segments for group g, emitted at
                j=1/2/3 of group g+1 so each cross-engine hop hides behind
                a full tile of independent work in the strict-FIFO queues.
                accq comes straight off uT_ps (Square of the scaled value,
                free-dim accumulated); the bf16 transpose of u feeds only
                the l-reductions (sum for accs2, max for accm) -- bf16
                rounding of u shifts the final loss by ~1e-5 relative."""
                state = {}

                def emit1():  # ACT: accq + u_sb(bf16)
                    usq = small.tile([2 * M, 2, P], F32)
                    nc.scalar.activation(
                        out=usq[:],
                        in_=uT_ps[:],
                        func=AF.Square,
                        scale=capinv_t[0 : 2 * M, 0:1],
                        accum_out=accq[0 : 2 * M, g : g + 1],
                    )
                    u_sb = usb.tile([2 * M, 2, P], BF16)
                    nc.scalar.activation(
                        out=u_sb[:],
                        in_=uT_ps[:],
                        func=AF.Copy,
                        scale=capinv_t[0 : 2 * M, 0:1],
                    )
                    state["u_sb"] = u_sb

                def emit2():  # PE: bf16 transposes back to [b, (s,j,rr,l)]
                    u2_ps = u2p.tile([P, 2, 2 * M], BF16)
                    for s in range(2):
                        nc.tensor.transpose(
                            out=u2_ps[:, s, :],
                            in_=state["u_sb"][:, s, :],
                            identity=identst_t[0 : 2 * M, 0 : 2 * M],
                        )
                    state["u2"] = u2_ps

                def emit3():  # DVE reduces + ACT accumulations
                    u2v = state["u2"][:].rearrange("p s (m l) -> p (s m) l", l=L)
                    s8 = small.tile([P, G * RR], F32)
                    nc.vector.reduce_sum(out=s8[:], in_=u2v, axis=X.X)
                    m8 = small.tile([P, G * RR], F32)
                    nc.vector.reduce_max(out=m8[:], in_=u2v, axis=X.X)
                    sq8 = small.tile([P, G * RR], F32)
                    nc.scalar.activation(
                        out=sq8[:],
                        in_=s8[:],
                        func=AF.Square,
                        accum_out=accs2[:, g : g + 1],
                    )
                    md = small.tile([P, G * RR], F32)
                    nc.scalar.activation(
                        out=md[:],
                        in_=m8[:],
                        func=AF.Copy,
                        accum_out=accm[:, g : g + 1],
                    )

                return [emit1, emit2, emit3]

            loop_cm = (
                tc.For_i(0, loop, 1, staggered_reset=stagger)
                if loop is not None
                else None
            )
            if loop_cm is not None:
                loop_cm.__enter__()
            pending_stats = []
            for rep in range(repeat):
                for g in range(ng):
                    gi = g % (n_tiles // G)
                    pred_g = io.tile([P, G, TW], F32)
                    dem_g = io.tile([P, G, RR * D], F32)
                    if "dma" in stages:
                        if pred_split:
                            half = GROWS // 2
                            for h, eng in enumerate((nc.sync, nc.scalar)):
                                lo = gi * GROWS + h * half
                                eng.dma_start(
                                    pred_g[:, 2 * h : 2 * (h + 1), :],
                                    pred_d.ap()[lo : lo + half, :].rearrange(
                                        "(r p rr) t -> p r (rr t)", p=P, rr=RR
                                    ),
                                )
                        else:
                            nc.sync.dma_start(
                                pred_g[:],
                                pred_d.ap()[
                                    gi * GROWS : (gi + 1) * GROWS, :
                                ].rearrange("(r p rr) t -> p r (rr t)", p=P, rr=RR),
                            )
                        dem_eng = {
                            "sync": nc.sync,
                            "scalar": nc.scalar,
                            "gpsimd": nc.gpsimd,
                        }[dem_q]
                        dem_eng.dma_start(
                            dem_g[:],
                            dem_d.ap()[gi * GROWS : (gi + 1) * GROWS, :].rearrange(
                                "(r p rr) d -> p r (rr d)", p=P, rr=RR
                            ),
                        )
                    uT_ps = uTp.tile([2 * M, 2, P], F32)
                    for j in range(G):
                        if "mul" not in stages:
                            continue
                        tt = work.tile([P, TW], TDT)
                        nc.vector.tensor_tensor(
                            out=tt[:].rearrange("p (e k) -> p e k", k=8),
                            in0=pred_g[:, j, :].rearrange("p (e k) -> p e k", k=8),
                            in1=dem_g[:, j, :]
                            .unsqueeze(2)
                            .broadcast_to([P, RR * D, 8]),
                            op=mybir.AluOpType.mult,
                        )
                        if "trans" not in stages:
                            continue
                        # fp8 transpose mode writes with element step 2, so
                        # the PSUM chunks keep a [.., P, 2] layout (same bank
                        # footprint as bf16); the ACT evacuation repacks.
                        es = 2 if tt_dtype == "fp8" else 1
                        ttA_ps = tpsA.tile([P, NCA, P, es], TDT)
                        ttB_ps = tpsB.tile([P, NCB, P, es], TDT)
                        for c in range(NCH):
                            dst = (
                                ttA_ps[:, c, :, 0]
                                if c < NCA
                                else ttB_ps[:, c - NCA, :, 0]
                            )
                            nc.tensor.transpose(
                                out=dst,
                                in_=tt[:, CH_OFF[c] : CH_OFF[c] + P],
                                identity=ident_t[:],
                            )
                        if pair_scat:
                            # two consecutive tiles share one evac tile; the
                            # scatter then streams both (256 moving cols per
                            # ldweights), halving PE scatter instructions.
                            # uT partition becomes (pair, rr, l), free becomes
                            # (tile-in-pair, b) -- stats only ever reduce over
                            # l and sum everything else, so the relabel is
                            # harmless.
                            if j % 2 == 0:
                                ttA = evac.tile([P, NCA, 2, P], TDT)
                                ttB = evac.tile([P, NCB, 2, P], TDT)
                                pair = (ttA, ttB)
                            ttA, ttB = pair
                            nc.scalar.copy(
                                out=ttA[:, :, j % 2, :], in_=ttA_ps[:, :, :, 0]
                            )
                            nc.scalar.copy(
                                out=ttB[:, :, j % 2, :], in_=ttB_ps[:, :, :, 0]
                            )
                            if "scat" not in stages:
                                continue
                            if j % 2 == 1:
                                k = j // 2
                                for c in range(NCH):
                                    src = (
                                        ttA[:, c, :, :]
                                        if c < NCA
                                        else ttB[:, c - NCA, :, :]
                                    )
                                    nc.tensor.matmul(
                                        out=uT_ps[M * k : M * (k + 1), :, :],
                                        lhsT=mask_t[:, c, :],
                                        rhs=src,
                                        start=(c == 0),
                                        stop=(c == NCH - 1),
                                    )
                        else:
                            ttA = evac.tile([P, NCA, P], TDT)
                            nc.scalar.copy(out=ttA[:], in_=ttA_ps[:, :, :, 0])
                            ttB = evac.tile([P, NCB, P], TDT)
                            nc.scalar.copy(out=ttB[:], in_=ttB_ps[:, :, :, 0])
                            if "scat" not in stages:
                                continue
                            jp, js = j % 2, j // 2
                            for c in range(NCH):
                                src = (
                                    ttA[:, c, :] if c < NCA else ttB[:, c - NCA, :]
                                )
                                nc.tensor.matmul(
                                    out=uT_ps[M * jp : M * (jp + 1), js, :],
                                    lhsT=mask_t[:, c, :],
                                    rhs=src,
                                    start=(c == 0),
                                    stop=(c == NCH - 1),
                                )
                        for _ in range(spread.count(j)):
                            if pending_stats:
                                pending_stats.pop(0)()
                        while pending_stats and j == G - 1:
                            pending_stats.pop(0)()
                    # --- stats for this group of G tiles: segments run at
                    # j=1/2/3 of the next group (see make_stats docstring) ---
                    if "stats" not in stages:
                        continue
                    assert not pending_stats
                    pending_stats = make_stats(uT_ps, g)
            for f in pending_stats:
                f()
            pending_stats = []
            if loop_cm is not None:
                loop_cm.__exit__(None, None, None)
            nc.sync.dma_start(out_d.ap()[0], accq[:])
            nc.sync.dma_start(out_d.ap()[1], accs2[:])
            nc.sync.dma_start(out_d.ap()[2], accm[:])
    _split_multi_waits(nc)
    return nc


def make_constants(tunnel_to_link, link_capacities):
    t2l = np.asarray(tunnel_to_link).astype(np.int64).ravel()
    cap = np.asarray(link_capacities, dtype=np.float32).ravel()
    # mask[k, c, rr*L + l]: chunk c covers free idx f = CH_OFF[c] + k,
    # f = rr*T + t; one-hot into (rr, link(t)).  The last chunk overlaps
    # chunk NCH-2 by 80 columns; overlap rows stay zero so those tunnels
    # are only counted once.
    mask = np.zeros((P, NCH, M), dtype=np.float32)
    for c in range(NCH):
        k_lo = 0 if c < NCH - 1 else (NCH - 1) * P - CH_OFF[c]
        for k in range(k_lo, P):
            f = CH_OFF[c] + k
            rr, t = divmod(f, T)
            mask[k, c, rr * L + int(t2l[t])] = 1.0
    mask = mask.reshape(P, NCH * M)
    ident = np.eye(P, dtype=np.float32)
    # uT partition p = 32j'' + 16rr + l  ->  l = p % 16
    capinv = (1.0 / (cap + 1e-8)).astype(np.float32)
    capinv_col = capinv[np.arange(P) % L][:, None].copy()
    return mask, ident, capinv_col


def _to_bf16(a):
    import ml_dtypes

    return a.astype(ml_dtypes.bfloat16)


def _to_fp8(a):
    import ml_dtypes

    return a.astype(ml_dtypes.float8_e4m3fn)


def run_cores(nc, pred, dem, mask, ident, capinv, tt_dtype="bf16", **kw):
    pred = np.ascontiguousarray(np.asarray(pred, dtype=np.float32))
    dem = np.ascontiguousarray(np.asarray(dem, dtype=np.float32))
    conv = _to_fp8 if tt_dtype == "fp8" else _to_bf16
    mask_c = conv(mask)
    ident_c = conv(ident)
    identst = _to_bf16(ident)
    capinv = np.asarray(capinv, dtype=np.float32)
    in_maps = []
    for i in range(N_CORES):
        in_maps.append(
            {
                "pred": pred[i * ROWS : (i + 1) * ROWS],
                "dem": dem[i * ROWS : (i + 1) * ROWS],
                "mask": mask_c,
                "ident": ident_c,
                "identst": identst,
                "capinv": capinv,
            }
        )
    return run_bass_kernel_spmd(nc, in_maps, core_ids=list(range(N_CORES)), **kw)


def combine_partials(partials_list):
    q = s2 = m = 0.0
    for p in partials_list:
        p = np.asarray(p, dtype=np.float64)
        q += p[0].sum()
        s2 += p[1].sum()
        m += p[2].sum()
    var_mean = (q - s2 / L) / (L - 1) / B
    return var_mean + 0.5 * m / B


def kernel(pred_ratios, demands, tunnel_to_link, link_capacities):
    mask, ident, capinv = make_constants(tunnel_to_link, link_capacities)
    nc = build_kernel()
    res = run_cores(nc, pred_ratios, demands, mask, ident, capinv)
    loss = combine_partials([r["partials"] for r in res.results])
    return np.array(loss, dtype=np.float32)
